# revision 12
# baseline (speedup 1.0000x reference)
"""Trainium2 Bass kernel for the attention module:

    att_h  = h @ W_h2att.T + b_h2att             # [B, 512]
    dot    = tanh(p_att_feats + att_h[:, None])  # [B, 1024, 512]
    scores = dot @ w_alpha + b_alpha             # [B, 1024]
    weight = softmax(scores, axis=1)
    out    = einsum('bs,bsd->bd', weight, att_feats)  # [B, 2048]

Sharding: data-parallel over batch B=64 across 8 NeuronCores (8 per core).
Params are tiny and replicated. b_alpha is a softmax shift -> dropped.

v2f design (DMA-bound; ~40MB/core HBM read, 16 SDMA engines saturated):
  - att/W in bf16, p in fp8-e4m3 (host-cast). Measured rel err ~1.3e-2
    against the fp32 reference (budget 2e-2). Set P_FP8=False to fall
    back to bf16 p (~3.3e-3).
  - All input DMAs on the SP HWDGE ring; ring order starts the big att
    stream immediately (att(0) before the 2MB W load) so the SDMA
    engines never idle during the prologue.
  - scores: DVE add -> ACT tanh (bf16) -> DVE in-place mul by
    w_alpha -> DVE reduce_sum into score columns.
  - softmax without max-subtraction (|scores| <= ||w_alpha||_1 ~ 18):
    one ACT Exp per batch emits the bf16 weight tile (matmul lhsT
    layout) and f32 per-partition partial sums into a column of zall;
    Z-reduction and 1/Z normalization happen on the host (64 divides).
  - att_h row-broadcast via selector-matrix TensorE matmul.
  - weighted sum: per-b M=1 PSUM-accumulating bf16 matmuls (1MB att
    tiles; 512KB for the last batch to shrink the drain tail); acc-copy
    emission precedes the next batch's tanh so PSUM banks free fast.

s-index mapping (shared by p, scores, weights, att):
    s = g*512 + q*4 + j   (g: group 0..1, q: partition 0..127, j: 0..3)
"""

import numpy as np
import ml_dtypes

import concourse.bass as bass
import concourse.tile as tile
from concourse import bacc, mybir
from concourse.bass import ts
from concourse.bass_utils import run_bass_kernel_spmd

F32 = mybir.dt.float32
BF16 = mybir.dt.bfloat16
F8 = mybir.dt.float8e4

P_FP8 = False
P_DT = F8 if P_FP8 else BF16

B_LOC = 8       # batches per core
S = 1024        # attended positions
J = 4           # s per (group, partition)
NG = 2          # s groups
ST = NG * J     # 8 score columns
HID = 512
D = 2048
DT = D // 512   # 4 output column slices
K = 2048        # rnn_size (contraction for att_h)
KJ = 2
KG = K // (128 * KJ)  # 8 k-groups

_NC_CACHE = None


def build_kernel(att_bufs=12, p_bufs=6):
    nc = bacc.Bacc("TRN2", target_bir_lowering=False, debug=False, num_devices=8)

    p_d = nc.dram_tensor("p", [B_LOC, S, HID], P_DT, kind="ExternalInput")
    att_d = nc.dram_tensor("att", [B_LOC, S, D], BF16, kind="ExternalInput")
    hT_d = nc.dram_tensor("hT", [K, B_LOC], BF16, kind="ExternalInput")
    WT_d = nc.dram_tensor("WT", [K, HID], BF16, kind="ExternalInput")
    wab_d = nc.dram_tensor("wab", [128, HID], BF16, kind="ExternalInput")
    bias8_d = nc.dram_tensor("bias8", [B_LOC, HID], F32, kind="ExternalInput")
    sel_d = nc.dram_tensor("sel", [B_LOC, B_LOC * 128], BF16, kind="ExternalInput")
    out_d = nc.dram_tensor("out", [B_LOC, D], F32, kind="ExternalOutput")
    z_d = nc.dram_tensor("zall", [128, B_LOC], F32, kind="ExternalOutput")

    with tile.TileContext(nc) as tc:
        with (
            tc.tile_pool(name="consts", bufs=1) as consts,
            tc.tile_pool(name="singles", bufs=1) as singles,
            tc.tile_pool(name="ahbc", bufs=B_LOC) as ahbcpool,
            tc.tile_pool(name="pp", bufs=p_bufs) as ppool,
            tc.tile_pool(name="pb", bufs=3) as pbpool,
            tc.tile_pool(name="th", bufs=3) as thpool,
            tc.tile_pool(name="sct", bufs=3) as sctpool,
            tc.tile_pool(name="wgtp", bufs=3) as wgtpool,
            tc.tile_pool(name="row", bufs=2) as rowpool,
            tc.tile_pool(name="attp", bufs=att_bufs) as attpool,
            tc.tile_pool(name="ps_setup", bufs=1, space=bass.MemorySpace.PSUM) as ps_setup,
            tc.tile_pool(name="ps_bc", bufs=1, space=bass.MemorySpace.PSUM) as ps_bc,
            tc.tile_pool(name="ps_acc", bufs=6, space=bass.MemorySpace.PSUM) as ps_acc,
        ):
            p_r = [
                p_d[b].rearrange("(g q j) h -> g q j h", q=128, j=J)
                for b in range(B_LOC)
            ]
            att_r = [
                att_d[b].rearrange("(g q j) h -> g q j h", q=128, j=J)
                for b in range(B_LOC)
            ]

            att_tiles = {}

            def emit_att_dma(b):
                # 1MB tiles (j-pairs); 512KB single-j tiles for the last
                # batch so the final drain is finer-grained
                tiles = []
                js = 1 if b == B_LOC - 1 else 2
                for g in range(NG):
                    for half in range(J // js):
                        at = attpool.tile(
                            [128, js, D], BF16, name=f"at{b}_{g}_{half}", tag="at"
                        )
                        nc.sync.dma_start(
                            at[:], att_r[b][g][:, js * half : js * (half + 1), :]
                        )
                        tiles.append((g, half, js, at))
                att_tiles[b] = tiles

            def emit_p_dma(b):
                tiles = []
                for g in range(NG):
                    pt = ppool.tile([128, J, HID], P_DT, name=f"pt{b}_{g}", tag="pt")
                    nc.sync.dma_start(pt[:], p_r[b][g])
                    tiles.append(pt)
                return tiles

            # ---- ring order: tiny consts, att(0), p(0), then W ----
            ht_all = consts.tile([128, KG, KJ, B_LOC], BF16)
            nc.sync.dma_start(
                ht_all[:], hT_d.rearrange("(kg q j) h -> q kg j h", q=128, j=KJ)
            )
            sel = consts.tile([B_LOC, B_LOC * 128], BF16)
            nc.sync.dma_start(sel[:], sel_d[:])
            wab = consts.tile([128, HID], BF16)
            nc.sync.dma_start(wab[:], wab_d[:])
            bias8 = consts.tile([B_LOC, HID], F32)
            nc.sync.dma_start(bias8[:], bias8_d[:])

            emit_att_dma(0)
            p_tiles = {0: emit_p_dma(0)}

            wt_all = consts.tile([128, KG, KJ, HID], BF16)
            nc.sync.dma_start(
                wt_all[:], WT_d.rearrange("(kg q j) h -> q kg j h", q=128, j=KJ)
            )

            # ---- att_h = h @ W.T + b  ([8, 512]) ----
            atth_ps = ps_setup.tile([B_LOC, HID], F32)
            for kg in range(KG):
                for j in range(KJ):
                    nc.tensor.matmul(
                        atth_ps[:], ht_all[:, kg, j, :], wt_all[:, kg, j, :],
                        start=(kg == 0 and j == 0),
                        stop=(kg == KG - 1 and j == KJ - 1),
                    )
            A2 = singles.tile([B_LOC, HID], BF16)
            nc.vector.tensor_add(A2[:], atth_ps[:], bias8[:])

            # per-partition exp partial sums, one column per batch
            zall = singles.tile([128, B_LOC], F32)

            ahbc = [None] * B_LOC

            def emit_bcast(b):
                # broadcast att_h row b across 128 partitions: sel_b.T @ A2
                bc = ps_bc.tile([128, HID], F32, name=f"bc{b}", tag="bc")
                nc.tensor.matmul(
                    bc[:], sel[:, b * 128 : (b + 1) * 128], A2[:],
                    start=True, stop=True,
                )
                t = ahbcpool.tile([128, HID], BF16, name=f"ahbc{b}", tag="ahbc")
                nc.scalar.copy(t[:], bc[:])
                ahbc[b] = t

            wgtT = {}

            def emit_scores(b):
                sc_b = sctpool.tile([128, ST], F32, name=f"sc{b}", tag="sc")
                for g in range(NG):
                    pt = p_tiles[b][g]
                    pb = pbpool.tile([128, J, HID], BF16, name=f"pb{b}_{g}", tag="pb")
                    nc.vector.tensor_add(
                        pb[:], pt[:],
                        ahbc[b][:, None, :].broadcast_to((128, J, HID)),
                    )
                    th = thpool.tile([128, J, HID], BF16, name=f"th{b}_{g}", tag="th")
                    nc.scalar.activation(
                        th[:], pb[:], mybir.ActivationFunctionType.Tanh
                    )
                    nc.vector.tensor_mul(
                        th[:], th[:],
                        wab[:, None, :].broadcast_to((128, J, HID)),
                    )
                    nc.vector.reduce_sum(
                        sc_b[:, ts(g, J)], th[:], axis=mybir.AxisListType.X
                    )
                wgt = wgtpool.tile([128, ST], BF16, name=f"wgt{b}", tag="wgt")
                nc.scalar.activation(
                    wgt[:], sc_b[:], mybir.ActivationFunctionType.Exp,
                    accum_out=zall[:, b : b + 1],
                )
                wgtT[b] = wgt

            def emit_weighted(b):
                accs = [
                    ps_acc.tile([1, 512], F32, name=f"acc{b}_{d}", tag="acc")
                    for d in range(DT)
                ]
                for g, half, js, at in att_tiles[b]:
                    for u in range(js):
                        t = g * J + half * js + u
                        for d in range(DT):
                            nc.tensor.matmul(
                                accs[d][:],
                                wgtT[b][:, t : t + 1],
                                at[:, u, ts(d, 512)],
                                start=(t == 0),
                                stop=(t == ST - 1),
                            )
                rowbuf = rowpool.tile([1, D], F32, name=f"row{b}", tag="row")
                for d in range(DT):
                    nc.scalar.copy(rowbuf[0:1, ts(d, 512)], accs[d][:])
                nc.scalar.dma_start(out_d[b : b + 1, :], rowbuf[:])

            # prologue: scores for b=0,1 ahead of the weighted stream
            emit_bcast(0)
            emit_scores(0)
            p_tiles[1] = emit_p_dma(1)
            emit_bcast(1)
            emit_scores(1)
            for b in range(B_LOC):
                if b + 1 < B_LOC:
                    emit_att_dma(b + 1)
                emit_weighted(b)
                if b + 2 < B_LOC:
                    p_tiles[b + 2] = emit_p_dma(b + 2)
                    emit_bcast(b + 2)
                    emit_scores(b + 2)

            nc.scalar.dma_start(z_d[:], zall[:])

    nc.compile()
    return nc


def _in_maps(h, att_feats, p_att_feats, W_h2att, b_h2att, w_alpha):
    bf = ml_dtypes.bfloat16
    p_np = ml_dtypes.float8_e4m3fn if P_FP8 else bf
    att_bf = np.ascontiguousarray(att_feats).astype(bf)
    p_q = np.ascontiguousarray(p_att_feats).astype(p_np)
    WT = np.ascontiguousarray(W_h2att.T).astype(bf)
    wab = np.ascontiguousarray(
        np.broadcast_to(w_alpha.astype(np.float32), (128, HID))
    ).astype(bf)
    bias8 = np.ascontiguousarray(
        np.broadcast_to(b_h2att.astype(np.float32), (B_LOC, HID))
    )
    sel = np.kron(
        np.eye(B_LOC, dtype=np.float32), np.ones((1, 128), dtype=np.float32)
    ).astype(bf)
    maps = []
    for c in range(8):
        sl = slice(c * B_LOC, (c + 1) * B_LOC)
        maps.append(
            {
                "p": np.ascontiguousarray(p_q[sl]),
                "att": np.ascontiguousarray(att_bf[sl]),
                "hT": np.ascontiguousarray(h[sl].T.astype(bf)),
                "WT": WT,
                "wab": wab,
                "bias8": bias8,
                "sel": sel,
            }
        )
    return maps


def kernel(h, att_feats, p_att_feats, W_h2att, b_h2att, w_alpha, b_alpha):
    global _NC_CACHE
    h = np.asarray(h)
    att_feats = np.asarray(att_feats)
    p_att_feats = np.asarray(p_att_feats)
    W_h2att = np.asarray(W_h2att)
    b_h2att = np.asarray(b_h2att)
    w_alpha = np.asarray(w_alpha)
    if _NC_CACHE is None:
        _NC_CACHE = build_kernel()
    nc = _NC_CACHE
    maps = _in_maps(h, att_feats, p_att_feats, W_h2att, b_h2att, w_alpha)
    res = run_bass_kernel_spmd(nc, maps, core_ids=list(range(8)))
    outs = []
    for c in range(8):
        row = res.results[c]["out"]                     # [8, 2048] unnormalized
        z = res.results[c]["zall"].sum(axis=0)          # [8]
        outs.append(row / z[:, None])
    return np.concatenate(outs, axis=0).astype(np.float32)
